# revision 21
# baseline (speedup 1.0000x reference)
"""Trainium2 Bass kernel for nn_ActorNetwork (gnn_message_passing).

Pure data-parallel across 8 NeuronCores: each core processes 8192 of the
65536 batch rows; small weights are replicated.

v2 layout: feature-major throughout, with the attention computed in a fused
(neighbor-pair, macro-column) layout that never transposes i2 back to
batch-major:
  - i1/i2 as 4 wide matmuls each (neighbor-pairs packed on partitions,
    (o,b) = 512 macro columns streamed).
  - score_n[b] = sum_d i2 * q2 via one elementwise multiply (q2 produced
    partition-duplicated for free by a duplicated-column Wq*Wk^T stationary)
    and per-pair ones-matmul partition reductions, accumulated in PSUM on
    top of the PE-transposed -1e30 mask rows -> masked scores directly.
  - softmax over an [8, 512] tile (exp on Act, denominator via ones-matmul,
    reciprocal-dup via 1x8 matmul, one multiply -> alpha).
  - alpha broadcast back to the (r,d) partition layout via 4 selector
    matmuls; weighted i2 (cmul) feeds h1 directly through a row-duplicated
    Wvc stationary -- the attention output is never materialized.
Elementwise/copy work is spread across DVE / Act / Pool; all wide moving
operands are bf16 (DVE 2x mode, PE 1 cyc/col).
"""

import os

import numpy as np
import ml_dtypes

import concourse.bass as bass
import concourse.tile as tile
from concourse import bacc
from concourse import mybir
from concourse.bass_utils import run_bass_kernel_spmd
from concourse.masks import make_identity

F32 = mybir.dt.float32
F32R = mybir.dt.float32r
BF16 = mybir.dt.bfloat16

N_CORES = 8
B_FULL = 65536
RPC = B_FULL // N_CORES        # rows per core = 8192
MACRO = 512                    # batch rows per macro tile
P = 128

Relu = mybir.ActivationFunctionType.Relu
Tanh = mybir.ActivationFunctionType.Tanh
Exp = mybir.ActivationFunctionType.Exp
Alu = mybir.AluOpType
AX = mybir.AxisListType


def build(rpc=RPC, macro=MACRO):
    nm = rpc // macro          # macro tiles per core
    nb = macro // P            # 128-row blocks per macro tile

    nc = bacc.Bacc()

    s0 = nc.declare_dram_parameter("state0", [rpc, 6], F32R, isOutput=False)
    s1 = nc.declare_dram_parameter("state1", [rpc, 1024], F32, isOutput=False)
    s2 = nc.declare_dram_parameter("state2", [rpc, 8, 7], F32, isOutput=False)
    wg_d = nc.declare_dram_parameter("wg_bf", [1024, 64], BF16, isOutput=False)
    ws1_d = nc.declare_dram_parameter("ws1_blk", [56, 4, 128], BF16, isOutput=False)
    ws2_d = nc.declare_dram_parameter("ws2_blk", [128, 128], BF16, isOutput=False)
    w0_d = nc.declare_dram_parameter("w0", [6, 64], F32R, isOutput=False)
    wqk_d = nc.declare_dram_parameter("wqk_dup", [64, 128], BF16, isOutput=False)
    wc1a_d = nc.declare_dram_parameter("wc1a", [128, 128], BF16, isOutput=False)
    wvc_d = nc.declare_dram_parameter("wvc_dup", [128, 128], BF16, isOutput=False)
    wc2_d = nc.declare_dram_parameter("wc2", [128, 128], BF16, isOutput=False)
    wc3_d = nc.declare_dram_parameter("wc3", [128, 2], BF16, isOutput=False)
    sel_d = nc.declare_dram_parameter("sel", [8, 4, 128], BF16, isOutput=False)
    ones2_d = nc.declare_dram_parameter("scoresel", [128, 4, 8], BF16, isOutput=False)
    ones8_d = nc.declare_dram_parameter("ones8", [8, 1], BF16, isOutput=False)
    ones18_d = nc.declare_dram_parameter("ones18", [1, 8], F32R, isOutput=False)
    jsum_d = nc.declare_dram_parameter("jsum", [56, 8], BF16, isOutput=False)
    b0bg_d = nc.declare_dram_parameter("b0bg", [128, 1], F32, isOutput=False)
    bs1_d = nc.declare_dram_parameter("bs1_rep", [128, 1], F32, isOutput=False)
    bs2_d = nc.declare_dram_parameter("bs2_rep", [128, 1], F32, isOutput=False)
    bc1_d = nc.declare_dram_parameter("bc1", [128, 1], F32, isOutput=False)
    bc2_d = nc.declare_dram_parameter("bc2", [128, 1], F32, isOutput=False)
    bc3_d = nc.declare_dram_parameter("bc3", [2, 1], F32, isOutput=False)
    out_d = nc.declare_dram_parameter("out", [2, rpc], F32, isOutput=True)

    with tile.TileContext(nc) as tc:
        consts = tc.alloc_tile_pool(name="consts", bufs=1)
        s1stage_p = tc.alloc_tile_pool(name="s1stage", bufs=2)
        s1T_p = tc.alloc_tile_pool(name="s1T", bufs=2)
        s2_p = tc.alloc_tile_pool(name="s2", bufs=2)
        work_p = tc.alloc_tile_pool(name="work", bufs=3)
        blk_p = tc.alloc_tile_pool(name="blk", bufs=3)
        sm_p = tc.alloc_tile_pool(name="sm", bufs=3)
        psS = tc.alloc_tile_pool(name="psS", bufs=2, space="PSUM")
        psA = tc.alloc_tile_pool(name="psA", bufs=2, space="PSUM")
        psB = tc.alloc_tile_pool(name="psB", bufs=1, space="PSUM")
        psB2 = tc.alloc_tile_pool(name="psB2", bufs=1, space="PSUM")
        psC = tc.alloc_tile_pool(name="psC", bufs=2, space="PSUM")

        # ---- constants / weights to SBUF ----
        wg_sb = consts.tile([P, 8, 64], BF16)
        nc.sync.dma_start(wg_sb, wg_d.rearrange("(c p) m -> p c m", p=P))
        ws1_sb = consts.tile([56, 4, P], BF16)
        nc.sync.dma_start(ws1_sb, ws1_d[:, :, :])
        ws2_sb = consts.tile([P, P], BF16)
        nc.sync.dma_start(ws2_sb, ws2_d[:, :])
        w0_sb = consts.tile([6, 64], F32R)
        nc.sync.dma_start(w0_sb, w0_d[:, :])
        wqk_sb = consts.tile([64, 128], BF16)
        nc.sync.dma_start(wqk_sb, wqk_d[:, :])
        wc1a_sb = consts.tile([P, 128], BF16)
        nc.sync.dma_start(wc1a_sb, wc1a_d[:, :])
        wvc_sb = consts.tile([P, 128], BF16)
        nc.sync.dma_start(wvc_sb, wvc_d[:, :])
        wc2_sb = consts.tile([P, 128], BF16)
        nc.sync.dma_start(wc2_sb, wc2_d[:, :])
        wc3_sb = consts.tile([P, 2], BF16)
        nc.sync.dma_start(wc3_sb, wc3_d[:, :])
        sel_sb = consts.tile([8, 4, P], BF16)
        nc.sync.dma_start(sel_sb, sel_d[:, :, :])
        ones2_sb = consts.tile([P, 4, 8], BF16)
        nc.sync.dma_start(ones2_sb, ones2_d[:, :, :])
        ones8_sb = consts.tile([8, 1], BF16)
        nc.sync.dma_start(ones8_sb, ones8_d[:, :])
        ones18_sb = consts.tile([1, 8], F32R)
        nc.sync.dma_start(ones18_sb, ones18_d[:, :])
        jsum_sb = consts.tile([56, 8], BF16)
        nc.sync.dma_start(jsum_sb, jsum_d[:, :])

        b0bg_sb = consts.tile([P, 1], F32)
        nc.sync.dma_start(b0bg_sb, b0bg_d[:, :])
        bs1_sb = consts.tile([P, 1], F32)
        nc.sync.dma_start(bs1_sb, bs1_d[:, :])
        bs2_sb = consts.tile([P, 1], F32)
        nc.sync.dma_start(bs2_sb, bs2_d[:, :])
        bc1_sb = consts.tile([P, 1], F32)
        nc.sync.dma_start(bc1_sb, bc1_d[:, :])
        bc2_sb = consts.tile([P, 1], F32)
        nc.sync.dma_start(bc2_sb, bc2_d[:, :])
        bc3_sb = consts.tile([2, 1], F32)
        nc.sync.dma_start(bc3_sb, bc3_d[:, :])

        # state0^T loaded once for the whole core (tiny, strided DMA)
        s0T_sb = consts.tile([6, rpc], F32R)
        with nc.allow_non_contiguous_dma(reason="tiny state0 transpose load"):
            nc.sync.dma_start(s0T_sb, s0.rearrange("b f -> f b"))

        ident_f = consts.tile([P, P], F32)
        make_identity(nc, ident_f)
        ident_b = consts.tile([P, P], BF16)
        nc.vector.tensor_copy(ident_b, ident_f)

        out_sb = consts.tile([2, rpc], F32)

        def emit_F1(ms):
            """Loads + state2 pack/transpose + mask rows."""
            row0 = ms * macro
            s1_stage = s1stage_p.tile([P, nb, 1024], BF16, tag="s1stage")
            nc.gpsimd.dma_start(
                s1_stage,
                s1[row0 : row0 + macro, :].rearrange("(o p) f -> p o f", p=P),
            )
            s2_t = s2_p.tile([P, nb, 8, 7], F32, tag="s2")
            nc.sync.dma_start(
                s2_t,
                s2[row0 : row0 + macro, :, :].rearrange("(o p) n j -> p o n j", p=P),
            )
            s2c = sm_p.tile([P, nb, 56], BF16, tag="s2c")
            nc.gpsimd.tensor_copy(s2c, s2_t.rearrange("p o n j -> p o (n j)"))
            s2T_ps = psS.tile([P, nb, P], BF16, tag="psS")
            for o in range(nb):
                nc.tensor.transpose(s2T_ps[0:56, o, :], s2c[:, o, :], ident_b)
            s2T_sb = sm_p.tile([56, nb, P], BF16, tag="s2T")
            nc.vector.tensor_copy(s2T_sb, s2T_ps[0:56])

            nmsum_ps = psC.tile([8, macro], F32, tag="psC")
            nc.tensor.matmul(nmsum_ps, jsum_sb, s2T_sb, start=True, stop=True)
            nm_sb = sm_p.tile([8, macro], BF16, tag="nm")
            nc.vector.tensor_scalar(
                nm_sb, nmsum_ps, 0.0, -1e30, Alu.is_equal, Alu.mult
            )
            return dict(row0=row0, s1_stage=s1_stage, s2T_sb=s2T_sb, nm_sb=nm_sb)

        def emit_F2(st):
            """state1 transposes + env/own + q2."""
            row0 = st["row0"]; s1_stage = st["s1_stage"]
            s1T = s1T_p.tile([P, 8, nb, P], BF16, tag="s1T")
            for o in range(nb):
                t_ps = psS.tile([P, 8, P], BF16, tag="psS")
                for c in range(8):
                    nc.tensor.transpose(
                        t_ps[:, c, :],
                        s1_stage[:, o, c * P : (c + 1) * P],
                        ident_b,
                    )
                dst = s1T[:, :, o, :]
                if o % 2 == 0:
                    nc.vector.tensor_copy(dst, t_ps)
                else:
                    nc.scalar.copy(dst, t_ps)

            eo_ps = psB.tile([P, macro], F32, tag="psB")
            nc.tensor.matmul(
                eo_ps[0:64, :], w0_sb, s0T_sb[:, row0 : row0 + macro],
                start=True, stop=True,
            )
            for c in range(8):
                nc.tensor.matmul(
                    eo_ps[64:128, :], wg_sb[:, c], s1T[:, c],
                    start=(c == 0), stop=(c == 7), tile_position=(0, 64),
                )
            concatA = work_p.tile([P, macro], BF16, tag="concatA")
            nc.scalar.activation(concatA, eo_ps, Relu, bias=b0bg_sb)

            q2_ps = psB.tile([P, macro], F32, tag="psB")
            nc.tensor.matmul(q2_ps, wqk_sb, concatA[0:64, :], start=True, stop=True)
            q2_sb = work_p.tile([P, macro], BF16, tag="q2")
            nc.scalar.copy(q2_sb, q2_ps)
            st["concatA"] = concatA
            st["q2_sb"] = q2_sb

        def emit_F3(st):
            """i1, i2, qk."""
            s2T_sb = st["s2T_sb"]; q2_sb = st["q2_sb"]
            i1_sb = blk_p.tile([P, 4, macro], BF16, tag="i1")
            for m in range(4):
                i1_ps = psA.tile([P, macro], F32, tag="psA")
                nc.tensor.matmul(i1_ps, ws1_sb[:, m], s2T_sb, start=True, stop=True)
                dst = i1_sb[:, m, :]
                if m in (0, 3):
                    nc.vector.tensor_scalar(dst, i1_ps, bs1_sb, 0.0, Alu.add, Alu.max)
                else:
                    nc.scalar.activation(dst, i1_ps, Relu, bias=bs1_sb)

            i2_sb = blk_p.tile([P, 4, macro], BF16, tag="i2")
            for m in range(4):
                i2_ps = psA.tile([P, macro], F32, tag="psA")
                nc.tensor.matmul(i2_ps, ws2_sb, i1_sb[:, m, :], start=True, stop=True)
                dst = i2_sb[:, m, :]
                if m in (1, 2):
                    nc.vector.tensor_scalar(dst, i2_ps, bs2_sb, 0.0, Alu.add, Alu.max)
                else:
                    nc.scalar.activation(dst, i2_ps, Relu, bias=bs2_sb)

            qk_sb = blk_p.tile([P, 4, macro], BF16, tag="qk")
            nc.vector.tensor_tensor(
                qk_sb[:, 0:2, :], i2_sb[:, 0:2, :],
                q2_sb[:, None, :].to_broadcast((P, 2, macro)),
                Alu.mult,
            )
            nc.gpsimd.tensor_tensor(
                qk_sb[:, 2:4, :], i2_sb[:, 2:4, :],
                q2_sb[:, None, :].to_broadcast((P, 2, macro)),
                Alu.mult,
            )
            st["i2_sb"] = i2_sb
            st["qk_sb"] = qk_sb

        def emit_B1(st):
            """Scores + softmax -> alpha."""
            qk_sb = st["qk_sb"]; nm_sb = st["nm_sb"]
            sc_ps = psC.tile([8, macro], F32, tag="psC")
            for m in range(4):
                nc.tensor.matmul(
                    sc_ps, ones2_sb[:, m, :], qk_sb[:, m, :],
                    start=(m == 0), stop=(m == 3),
                )
            nc.vector.tensor_tensor(sc_ps, sc_ps, nm_sb, Alu.add)

            p8_sb = sm_p.tile([8, macro], BF16, tag="p8")
            nc.scalar.activation(p8_sb, sc_ps, Exp, scale=0.125)
            den_ps = psC.tile([1, macro], F32, tag="psC")
            nc.tensor.matmul(den_ps, ones8_sb, p8_sb, start=True, stop=True)
            rs_sb = sm_p.tile([1, macro], F32R, tag="rs")
            with nc.allow_low_precision(reason="f32r reciprocal, 19-bit ok"):
                nc.vector.reciprocal(rs_sb, den_ps)
            dup_ps = psC.tile([8, macro], F32, tag="psC")
            nc.tensor.matmul(dup_ps, ones18_sb, rs_sb, start=True, stop=True)
            alpha_sb = sm_p.tile([8, macro], BF16, tag="alpha")
            nc.vector.tensor_tensor(alpha_sb, p8_sb, dup_ps, Alu.mult)
            st["alpha_sb"] = alpha_sb

        def emit_B2(st):
            """Weighted i2 + head + tanh."""
            row0 = st["row0"]; concatA = st["concatA"]
            i2_sb = st["i2_sb"]; alpha_sb = st["alpha_sb"]

            cmul_sb = blk_p.tile([P, 4, macro], BF16, tag="cmul")
            abc_sb = sm_p.tile([P, 2, macro], BF16, tag="abc")
            for m in range(4):
                abc_ps = psA.tile([P, macro], F32, tag="psA")
                nc.tensor.matmul(abc_ps, sel_sb[:, m], alpha_sb, start=True, stop=True)
                if m < 2:
                    nc.vector.tensor_tensor(
                        cmul_sb[:, m, :], i2_sb[:, m, :], abc_ps, Alu.mult
                    )
                else:
                    nc.scalar.copy(abc_sb[:, m - 2, :], abc_ps)
            nc.gpsimd.tensor_tensor(
                cmul_sb[:, 2:4, :], i2_sb[:, 2:4, :], abc_sb, Alu.mult
            )

            h1_ps = psB2.tile([P, macro], F32, tag="psB2")
            nc.tensor.matmul(h1_ps, wc1a_sb, concatA, start=True, stop=False)
            for m in range(4):
                nc.tensor.matmul(
                    h1_ps, wvc_sb, cmul_sb[:, m, :],
                    start=False, stop=(m == 3),
                )
            h1_sb = work_p.tile([P, macro], BF16, tag="h1")
            nc.scalar.activation(h1_sb, h1_ps, Relu, bias=bc1_sb)

            h2_ps = psB2.tile([P, macro], F32, tag="psB2")
            nc.tensor.matmul(h2_ps, wc2_sb, h1_sb, start=True, stop=True)
            h2_sb = work_p.tile([P, macro], BF16, tag="h2")
            nc.vector.tensor_scalar(h2_sb, h2_ps, bc2_sb, 0.0, Alu.add, Alu.max)

            o_ps = psB2.tile([2, macro], F32, tag="psB2")
            nc.tensor.matmul(o_ps, wc3_sb, h2_sb, start=True, stop=True)
            nc.scalar.activation(
                out_sb[:, row0 : row0 + macro], o_ps, Tanh, bias=bc3_sb
            )

        # software pipeline, fine-grained: each engine's in-order stream
        # alternates between macro m's latency-critical back half and macro
        # m+1's bulk front half, so dependency stalls are filled.
        prev = None
        for ms in range(nm):
            cur = emit_F1(ms)
            if prev is not None:
                emit_B1(prev)
            emit_F2(cur)
            if prev is not None:
                emit_B2(prev)
            emit_F3(cur)
            prev = cur
        emit_B1(prev)
        emit_B2(prev)

        nc.sync.dma_start(out_d[:, :], out_sb)

        for _pool in (psC, psB2, psB, psA, psS, sm_p, blk_p, work_p, s2_p, s1T_p,
                      s1stage_p, consts):
            _pool.release()

    return nc


def prepare_in_maps(inputs):
    bf = ml_dtypes.bfloat16
    f32 = np.float32

    def a(x, dt=f32):
        return np.ascontiguousarray(np.asarray(x), dtype=dt)

    W0 = a(inputs["W0"]); Wg = a(inputs["Wg"])
    Ws1 = a(inputs["Ws1"]); Ws2 = a(inputs["Ws2"])
    Wq = a(inputs["Wq"]); Wk = a(inputs["Wk"]); Wv = a(inputs["Wv"])
    Wc1 = a(inputs["Wc1"]); Wc2 = a(inputs["Wc2"]); Wc3 = a(inputs["Wc3"])

    wqk = Wq @ Wk.T                                   # [64, 64]
    wqk_dup = np.concatenate([wqk, wqk], axis=1)      # [64, 128]
    wvc = Wv @ Wc1[128:192, :]                        # [64, 128]
    wvc_dup = np.concatenate([wvc, wvc], axis=0)      # [128, 128]

    # ws1_blk[7n+j, m, 64r+d] = Ws1[j, d] if n == 2m+r
    ws1_blk = np.zeros((56, 4, 128), dtype=f32)
    sel = np.zeros((8, 4, 128), dtype=f32)
    for n in range(8):
        m, r = n // 2, n % 2
        ws1_blk[7 * n : 7 * n + 7, m, 64 * r : 64 * r + 64] = Ws1
        sel[n, m, 64 * r : 64 * r + 64] = 1.0
    ws2_blk = np.zeros((128, 128), dtype=f32)
    ws2_blk[0:64, 0:64] = Ws2
    ws2_blk[64:128, 64:128] = Ws2

    # scoresel[64r+d, m, n] = 1 iff n == 2m+r (score partition reduce)
    scoresel = np.zeros((128, 4, 8), dtype=f32)
    for n in range(8):
        m, r = n // 2, n % 2
        scoresel[64 * r : 64 * r + 64, m, n] = 1.0
    ones8 = np.ones((8, 1), dtype=f32)
    ones18 = np.ones((1, 8), dtype=f32)
    # jsum[7n+j, n'] = 1 iff n == n'  (per-neighbor feature sums for the mask)
    jsum = np.zeros((56, 8), dtype=f32)
    for n in range(8):
        jsum[7 * n : 7 * n + 7, n] = 1.0

    def col(x):
        return np.ascontiguousarray(np.asarray(x, dtype=f32).reshape(-1, 1))

    b0bg = np.concatenate([col(inputs["b0"]), col(inputs["bg"])], axis=0)
    bs1_rep = np.concatenate([col(inputs["bs1"])] * 2, axis=0)
    bs2_rep = np.concatenate([col(inputs["bs2"])] * 2, axis=0)

    state0 = a(inputs["state0"]); state1 = a(inputs["state1"])
    state2 = a(inputs["state2"])

    shared = {
        "wg_bf": a(Wg, bf),
        "ws1_blk": a(ws1_blk, bf),
        "ws2_blk": a(ws2_blk, bf),
        "w0": W0,
        "wqk_dup": a(wqk_dup, bf),
        "wc1a": a(Wc1[0:128, :], bf),
        "wvc_dup": a(wvc_dup, bf),
        "wc2": a(Wc2, bf),
        "wc3": a(Wc3, bf),
        "sel": a(sel, bf),
        "scoresel": a(scoresel, bf),
        "ones8": a(ones8, bf),
        "ones18": ones18,
        "jsum": a(jsum, bf),
        "b0bg": b0bg,
        "bs1_rep": bs1_rep,
        "bs2_rep": bs2_rep,
        "bc1": col(inputs["bc1"]),
        "bc2": col(inputs["bc2"]),
        "bc3": col(inputs["bc3"]),
    }
    in_maps = []
    for i in range(N_CORES):
        m = dict(shared)
        sl = slice(i * RPC, (i + 1) * RPC)
        m["state0"] = state0[sl]
        m["state1"] = state1[sl]
        m["state2"] = state2[sl]
        in_maps.append(m)
    return in_maps


_NC_CACHE = {}


def get_nc():
    if "nc" not in _NC_CACHE:
        nc = build()
        nc.finalize()
        _NC_CACHE["nc"] = nc
    return _NC_CACHE["nc"]


def kernel(**inputs):
    nc = get_nc()
    in_maps = prepare_in_maps(inputs)
    trace = bool(int(os.environ.get("K_TRACE", "0")))
    try:
        res = run_bass_kernel_spmd(
            nc, in_maps, core_ids=list(range(N_CORES)), trace=trace
        )
    except ModuleNotFoundError:
        res = run_bass_kernel_spmd(nc, in_maps, core_ids=list(range(N_CORES)))
    if res.exec_time_ns is not None:
        print(f"HW exec time: {res.exec_time_ns} ns")
    parts = [np.asarray(res.results[i]["out"], dtype=np.float32).T for i in range(N_CORES)]
    return np.ascontiguousarray(np.concatenate(parts, axis=0))


# revision 22
# speedup vs baseline: 1.2274x; 1.2274x over previous
"""Trainium2 Bass kernel for nn_ActorNetwork (gnn_message_passing).

Pure data-parallel across 8 NeuronCores: each core processes 8192 of the
65536 batch rows; small weights are replicated.

v2 layout: feature-major throughout, with the attention computed in a fused
(neighbor-pair, macro-column) layout that never transposes i2 back to
batch-major:
  - i1/i2 as 4 wide matmuls each (neighbor-pairs packed on partitions,
    (o,b) = 512 macro columns streamed).
  - score_n[b] = sum_d i2 * q2 via one elementwise multiply (q2 produced
    partition-duplicated for free by a duplicated-column Wq*Wk^T stationary)
    and per-pair ones-matmul partition reductions, accumulated in PSUM on
    top of the PE-transposed -1e30 mask rows -> masked scores directly.
  - softmax over an [8, 512] tile (exp on Act, denominator via ones-matmul,
    reciprocal-dup via 1x8 matmul, one multiply -> alpha).
  - alpha broadcast back to the (r,d) partition layout via 4 selector
    matmuls; weighted i2 (cmul) feeds h1 directly through a row-duplicated
    Wvc stationary -- the attention output is never materialized.
Elementwise/copy work is spread across DVE / Act / Pool; all wide moving
operands are bf16 (DVE 2x mode, PE 1 cyc/col).
"""

import os

import numpy as np
import ml_dtypes

import concourse.bass as bass
import concourse.tile as tile
from concourse import bacc
from concourse import mybir
from concourse.bass_utils import run_bass_kernel_spmd
from concourse.masks import make_identity

F32 = mybir.dt.float32
F32R = mybir.dt.float32r
BF16 = mybir.dt.bfloat16

N_CORES = 8
B_FULL = 65536
RPC = B_FULL // N_CORES        # rows per core = 8192
MACRO = 512                    # batch rows per macro tile
P = 128

Relu = mybir.ActivationFunctionType.Relu
Tanh = mybir.ActivationFunctionType.Tanh
Exp = mybir.ActivationFunctionType.Exp
Alu = mybir.AluOpType
AX = mybir.AxisListType


def build(rpc=RPC, macro=MACRO):
    nm = rpc // macro          # macro tiles per core
    nb = macro // P            # 128-row blocks per macro tile

    nc = bacc.Bacc()

    s0 = nc.declare_dram_parameter("state0", [rpc, 6], F32R, isOutput=False)
    s1 = nc.declare_dram_parameter("state1", [rpc, 1024], F32, isOutput=False)
    s2 = nc.declare_dram_parameter("state2", [rpc, 8, 7], F32, isOutput=False)
    wg_d = nc.declare_dram_parameter("wg_bf", [1024, 64], BF16, isOutput=False)
    ws1_d = nc.declare_dram_parameter("ws1_blk", [56, 4, 128], BF16, isOutput=False)
    ws2_d = nc.declare_dram_parameter("ws2_blk", [128, 128], BF16, isOutput=False)
    w0_d = nc.declare_dram_parameter("w0", [6, 64], F32R, isOutput=False)
    wqk_d = nc.declare_dram_parameter("wqk_dup", [64, 128], BF16, isOutput=False)
    wc1a_d = nc.declare_dram_parameter("wc1a", [128, 128], BF16, isOutput=False)
    wvc_d = nc.declare_dram_parameter("wvc_dup", [128, 128], BF16, isOutput=False)
    wc2_d = nc.declare_dram_parameter("wc2", [128, 128], BF16, isOutput=False)
    wc3_d = nc.declare_dram_parameter("wc3", [128, 2], BF16, isOutput=False)
    sel_d = nc.declare_dram_parameter("sel", [8, 4, 128], BF16, isOutput=False)
    ones2_d = nc.declare_dram_parameter("scoresel", [128, 4, 8], BF16, isOutput=False)
    ones8_d = nc.declare_dram_parameter("ones8", [8, 1], BF16, isOutput=False)
    ones18_d = nc.declare_dram_parameter("ones18", [1, 8], F32R, isOutput=False)
    jsum_d = nc.declare_dram_parameter("jsum", [56, 8], BF16, isOutput=False)
    b0bg_d = nc.declare_dram_parameter("b0bg", [128, 1], F32, isOutput=False)
    bs1_d = nc.declare_dram_parameter("bs1_rep", [128, 1], F32, isOutput=False)
    bs2_d = nc.declare_dram_parameter("bs2_rep", [128, 1], F32, isOutput=False)
    bc1_d = nc.declare_dram_parameter("bc1", [128, 1], F32, isOutput=False)
    bc2_d = nc.declare_dram_parameter("bc2", [128, 1], F32, isOutput=False)
    bc3_d = nc.declare_dram_parameter("bc3", [2, 1], F32, isOutput=False)
    out_d = nc.declare_dram_parameter("out", [2, rpc], F32, isOutput=True)

    with tile.TileContext(nc) as tc:
        consts = tc.alloc_tile_pool(name="consts", bufs=1)
        s1stage_p = tc.alloc_tile_pool(name="s1stage", bufs=2)
        s1T_p = tc.alloc_tile_pool(name="s1T", bufs=2)
        s2_p = tc.alloc_tile_pool(name="s2", bufs=2)
        work_p = tc.alloc_tile_pool(name="work", bufs=3)
        blk_p = tc.alloc_tile_pool(name="blk", bufs=3)
        sm_p = tc.alloc_tile_pool(name="sm", bufs=3)
        psS = tc.alloc_tile_pool(name="psS", bufs=2, space="PSUM")
        psA = tc.alloc_tile_pool(name="psA", bufs=2, space="PSUM")
        psB = tc.alloc_tile_pool(name="psB", bufs=1, space="PSUM")
        psB2 = tc.alloc_tile_pool(name="psB2", bufs=1, space="PSUM")
        psC = tc.alloc_tile_pool(name="psC", bufs=2, space="PSUM")

        # ---- constants / weights to SBUF ----
        wg_sb = consts.tile([P, 8, 64], BF16)
        nc.sync.dma_start(wg_sb, wg_d.rearrange("(c p) m -> p c m", p=P))
        ws1_sb = consts.tile([56, 4, P], BF16)
        nc.sync.dma_start(ws1_sb, ws1_d[:, :, :])
        ws2_sb = consts.tile([P, P], BF16)
        nc.sync.dma_start(ws2_sb, ws2_d[:, :])
        w0_sb = consts.tile([6, 64], F32R)
        nc.sync.dma_start(w0_sb, w0_d[:, :])
        wqk_sb = consts.tile([64, 128], BF16)
        nc.sync.dma_start(wqk_sb, wqk_d[:, :])
        wc1a_sb = consts.tile([P, 128], BF16)
        nc.sync.dma_start(wc1a_sb, wc1a_d[:, :])
        wvc_sb = consts.tile([P, 128], BF16)
        nc.sync.dma_start(wvc_sb, wvc_d[:, :])
        wc2_sb = consts.tile([P, 128], BF16)
        nc.sync.dma_start(wc2_sb, wc2_d[:, :])
        wc3_sb = consts.tile([P, 2], BF16)
        nc.sync.dma_start(wc3_sb, wc3_d[:, :])
        sel_sb = consts.tile([8, 4, P], BF16)
        nc.sync.dma_start(sel_sb, sel_d[:, :, :])
        ones2_sb = consts.tile([P, 4, 8], BF16)
        nc.sync.dma_start(ones2_sb, ones2_d[:, :, :])
        ones8_sb = consts.tile([8, 1], BF16)
        nc.sync.dma_start(ones8_sb, ones8_d[:, :])
        ones18_sb = consts.tile([1, 8], F32R)
        nc.sync.dma_start(ones18_sb, ones18_d[:, :])
        jsum_sb = consts.tile([56, 8], BF16)
        nc.sync.dma_start(jsum_sb, jsum_d[:, :])

        b0bg_sb = consts.tile([P, 1], F32)
        nc.sync.dma_start(b0bg_sb, b0bg_d[:, :])
        bs1_sb = consts.tile([P, 1], F32)
        nc.sync.dma_start(bs1_sb, bs1_d[:, :])
        bs2_sb = consts.tile([P, 1], F32)
        nc.sync.dma_start(bs2_sb, bs2_d[:, :])
        bc1_sb = consts.tile([P, 1], F32)
        nc.sync.dma_start(bc1_sb, bc1_d[:, :])
        bc2_sb = consts.tile([P, 1], F32)
        nc.sync.dma_start(bc2_sb, bc2_d[:, :])
        bc3_sb = consts.tile([2, 1], F32)
        nc.sync.dma_start(bc3_sb, bc3_d[:, :])

        # state0^T loaded once for the whole core (tiny, strided DMA)
        s0T_sb = consts.tile([6, rpc], F32R)
        with nc.allow_non_contiguous_dma(reason="tiny state0 transpose load"):
            nc.sync.dma_start(s0T_sb, s0.rearrange("b f -> f b"))

        ident_f = consts.tile([P, P], F32)
        make_identity(nc, ident_f)
        ident_b = consts.tile([P, P], BF16)
        nc.vector.tensor_copy(ident_b, ident_f)

        out_sb = consts.tile([2, rpc], F32)

        def emit_F1(ms):
            """Loads + state2 pack/transpose + mask rows."""
            row0 = ms * macro
            s1_stage = s1stage_p.tile([P, nb, 1024], BF16, tag="s1stage")
            nc.gpsimd.dma_start(
                s1_stage,
                s1[row0 : row0 + macro, :].rearrange("(o p) f -> p o f", p=P),
            )
            s2_t = s2_p.tile([P, nb, 8, 7], F32, tag="s2")
            nc.sync.dma_start(
                s2_t,
                s2[row0 : row0 + macro, :, :].rearrange("(o p) n j -> p o n j", p=P),
            )
            s2c = sm_p.tile([P, nb, 56], BF16, tag="s2c")
            nc.gpsimd.tensor_copy(s2c, s2_t.rearrange("p o n j -> p o (n j)"))
            s2T_ps = psS.tile([P, nb, P], BF16, tag="psS")
            for o in range(nb):
                nc.tensor.transpose(s2T_ps[0:56, o, :], s2c[:, o, :], ident_b)
            s2T_sb = sm_p.tile([56, nb, P], BF16, tag="s2T")
            nc.vector.tensor_copy(s2T_sb, s2T_ps[0:56])

            nmsum_ps = psC.tile([8, macro], F32, tag="psC")
            nc.tensor.matmul(nmsum_ps, jsum_sb, s2T_sb, start=True, stop=True)
            nm_sb = sm_p.tile([8, macro], BF16, tag="nm")
            nc.vector.tensor_scalar(
                nm_sb, nmsum_ps, 0.0, -1e30, Alu.is_equal, Alu.mult
            )
            return dict(row0=row0, s1_stage=s1_stage, s2T_sb=s2T_sb, nm_sb=nm_sb)

        def emit_F2(st):
            """state1 transposes + env/own + q2."""
            row0 = st["row0"]; s1_stage = st["s1_stage"]
            s1T = s1T_p.tile([P, 8, nb, P], BF16, tag="s1T")
            for o in range(nb):
                t_ps = psS.tile([P, 8, P], BF16, tag="psS")
                for c in range(8):
                    nc.tensor.transpose(
                        t_ps[:, c, :],
                        s1_stage[:, o, c * P : (c + 1) * P],
                        ident_b,
                    )
                dst = s1T[:, :, o, :]
                if o % 2 == 0:
                    nc.vector.tensor_copy(dst, t_ps)
                else:
                    nc.scalar.copy(dst, t_ps)

            eo_ps = psB.tile([P, macro], F32, tag="psB")
            nc.tensor.matmul(
                eo_ps[0:64, :], w0_sb, s0T_sb[:, row0 : row0 + macro],
                start=True, stop=True,
            )
            for c in range(8):
                nc.tensor.matmul(
                    eo_ps[64:128, :], wg_sb[:, c], s1T[:, c],
                    start=(c == 0), stop=(c == 7), tile_position=(0, 64),
                )
            concatA = work_p.tile([P, macro], BF16, tag="concatA")
            nc.scalar.activation(concatA, eo_ps, Relu, bias=b0bg_sb)

            q2_ps = psB.tile([P, macro], F32, tag="psB")
            nc.tensor.matmul(q2_ps, wqk_sb, concatA[0:64, :], start=True, stop=True)
            q2_sb = work_p.tile([P, macro], BF16, tag="q2")
            nc.scalar.copy(q2_sb, q2_ps)
            st["concatA"] = concatA
            st["q2_sb"] = q2_sb

        def emit_F3(st):
            """i1, i2, qk."""
            s2T_sb = st["s2T_sb"]; q2_sb = st["q2_sb"]
            i1_sb = blk_p.tile([P, 4, macro], BF16, tag="i1")
            for m in range(4):
                i1_ps = psA.tile([P, macro], F32, tag="psA")
                nc.tensor.matmul(i1_ps, ws1_sb[:, m], s2T_sb, start=True, stop=True)
                dst = i1_sb[:, m, :]
                if m in (0, 3):
                    nc.vector.tensor_scalar(dst, i1_ps, bs1_sb, 0.0, Alu.add, Alu.max)
                else:
                    nc.scalar.activation(dst, i1_ps, Relu, bias=bs1_sb)

            i2_sb = blk_p.tile([P, 4, macro], BF16, tag="i2")
            for m in range(4):
                i2_ps = psA.tile([P, macro], F32, tag="psA")
                nc.tensor.matmul(i2_ps, ws2_sb, i1_sb[:, m, :], start=True, stop=True)
                dst = i2_sb[:, m, :]
                if m in (1, 2):
                    nc.vector.tensor_scalar(dst, i2_ps, bs2_sb, 0.0, Alu.add, Alu.max)
                else:
                    nc.scalar.activation(dst, i2_ps, Relu, bias=bs2_sb)

            qk_sb = blk_p.tile([P, 4, macro], BF16, tag="qk")
            nc.vector.tensor_tensor(
                qk_sb[:, 0:2, :], i2_sb[:, 0:2, :],
                q2_sb[:, None, :].to_broadcast((P, 2, macro)),
                Alu.mult,
            )
            nc.gpsimd.tensor_tensor(
                qk_sb[:, 2:4, :], i2_sb[:, 2:4, :],
                q2_sb[:, None, :].to_broadcast((P, 2, macro)),
                Alu.mult,
            )
            st["i2_sb"] = i2_sb
            st["qk_sb"] = qk_sb

        def emit_B1a(st):
            """Scores -> exp."""
            qk_sb = st["qk_sb"]; nm_sb = st["nm_sb"]
            sc_ps = psC.tile([8, macro], F32, tag="psC")
            for m in range(4):
                nc.tensor.matmul(
                    sc_ps, ones2_sb[:, m, :], qk_sb[:, m, :],
                    start=(m == 0), stop=(m == 3),
                )
            nc.vector.tensor_tensor(sc_ps, sc_ps, nm_sb, Alu.add)
            p8_sb = sm_p.tile([8, macro], BF16, tag="p8")
            nc.scalar.activation(p8_sb, sc_ps, Exp, scale=0.125)
            st["p8_sb"] = p8_sb

        def emit_B1b(st):
            """Softmax denominator -> alpha."""
            p8_sb = st["p8_sb"]
            den_ps = psC.tile([1, macro], F32, tag="psC")
            nc.tensor.matmul(den_ps, ones8_sb, p8_sb, start=True, stop=True)
            rs_sb = sm_p.tile([1, macro], F32R, tag="rs")
            with nc.allow_low_precision(reason="f32r reciprocal, 19-bit ok"):
                nc.vector.reciprocal(rs_sb, den_ps)
            dup_ps = psC.tile([8, macro], F32, tag="psC")
            nc.tensor.matmul(dup_ps, ones18_sb, rs_sb, start=True, stop=True)
            alpha_sb = sm_p.tile([8, macro], BF16, tag="alpha")
            nc.vector.tensor_tensor(alpha_sb, p8_sb, dup_ps, Alu.mult)
            st["alpha_sb"] = alpha_sb

        def emit_B2(st):
            """Weighted i2 + head + tanh."""
            row0 = st["row0"]; concatA = st["concatA"]
            i2_sb = st["i2_sb"]; alpha_sb = st["alpha_sb"]

            cmul_sb = blk_p.tile([P, 4, macro], BF16, tag="cmul")
            abc_sb = sm_p.tile([P, 2, macro], BF16, tag="abc")
            for m in range(4):
                abc_ps = psA.tile([P, macro], F32, tag="psA")
                nc.tensor.matmul(abc_ps, sel_sb[:, m], alpha_sb, start=True, stop=True)
                if m < 2:
                    nc.vector.tensor_tensor(
                        cmul_sb[:, m, :], i2_sb[:, m, :], abc_ps, Alu.mult
                    )
                else:
                    nc.scalar.copy(abc_sb[:, m - 2, :], abc_ps)
            nc.gpsimd.tensor_tensor(
                cmul_sb[:, 2:4, :], i2_sb[:, 2:4, :], abc_sb, Alu.mult
            )

            h1_ps = psB2.tile([P, macro], F32, tag="psB2")
            nc.tensor.matmul(h1_ps, wc1a_sb, concatA, start=True, stop=False)
            for m in range(4):
                nc.tensor.matmul(
                    h1_ps, wvc_sb, cmul_sb[:, m, :],
                    start=False, stop=(m == 3),
                )
            h1_sb = work_p.tile([P, macro], BF16, tag="h1")
            nc.scalar.activation(h1_sb, h1_ps, Relu, bias=bc1_sb)

            h2_ps = psB2.tile([P, macro], F32, tag="psB2")
            nc.tensor.matmul(h2_ps, wc2_sb, h1_sb, start=True, stop=True)
            h2_sb = work_p.tile([P, macro], BF16, tag="h2")
            nc.vector.tensor_scalar(h2_sb, h2_ps, bc2_sb, 0.0, Alu.add, Alu.max)

            o_ps = psB2.tile([2, macro], F32, tag="psB2")
            nc.tensor.matmul(o_ps, wc3_sb, h2_sb, start=True, stop=True)
            nc.scalar.activation(
                out_sb[:, row0 : row0 + macro], o_ps, Tanh, bias=bc3_sb
            )

        # software pipeline, fine-grained: each engine's in-order stream
        # alternates between macro m's latency-critical back half and macro
        # m+1's bulk front half, so dependency stalls are filled.
        prev = None
        for ms in range(nm):
            cur = emit_F1(ms)
            if prev is not None:
                emit_B1a(prev)
            emit_F2(cur)
            if prev is not None:
                emit_B1b(prev)
            emit_F3(cur)
            if prev is not None:
                emit_B2(prev)
            prev = cur
        emit_B1a(prev)
        emit_B1b(prev)
        emit_B2(prev)

        nc.sync.dma_start(out_d[:, :], out_sb)

        for _pool in (psC, psB2, psB, psA, psS, sm_p, blk_p, work_p, s2_p, s1T_p,
                      s1stage_p, consts):
            _pool.release()

    return nc


def prepare_in_maps(inputs):
    bf = ml_dtypes.bfloat16
    f32 = np.float32

    def a(x, dt=f32):
        return np.ascontiguousarray(np.asarray(x), dtype=dt)

    W0 = a(inputs["W0"]); Wg = a(inputs["Wg"])
    Ws1 = a(inputs["Ws1"]); Ws2 = a(inputs["Ws2"])
    Wq = a(inputs["Wq"]); Wk = a(inputs["Wk"]); Wv = a(inputs["Wv"])
    Wc1 = a(inputs["Wc1"]); Wc2 = a(inputs["Wc2"]); Wc3 = a(inputs["Wc3"])

    wqk = Wq @ Wk.T                                   # [64, 64]
    wqk_dup = np.concatenate([wqk, wqk], axis=1)      # [64, 128]
    wvc = Wv @ Wc1[128:192, :]                        # [64, 128]
    wvc_dup = np.concatenate([wvc, wvc], axis=0)      # [128, 128]

    # ws1_blk[7n+j, m, 64r+d] = Ws1[j, d] if n == 2m+r
    ws1_blk = np.zeros((56, 4, 128), dtype=f32)
    sel = np.zeros((8, 4, 128), dtype=f32)
    for n in range(8):
        m, r = n // 2, n % 2
        ws1_blk[7 * n : 7 * n + 7, m, 64 * r : 64 * r + 64] = Ws1
        sel[n, m, 64 * r : 64 * r + 64] = 1.0
    ws2_blk = np.zeros((128, 128), dtype=f32)
    ws2_blk[0:64, 0:64] = Ws2
    ws2_blk[64:128, 64:128] = Ws2

    # scoresel[64r+d, m, n] = 1 iff n == 2m+r (score partition reduce)
    scoresel = np.zeros((128, 4, 8), dtype=f32)
    for n in range(8):
        m, r = n // 2, n % 2
        scoresel[64 * r : 64 * r + 64, m, n] = 1.0
    ones8 = np.ones((8, 1), dtype=f32)
    ones18 = np.ones((1, 8), dtype=f32)
    # jsum[7n+j, n'] = 1 iff n == n'  (per-neighbor feature sums for the mask)
    jsum = np.zeros((56, 8), dtype=f32)
    for n in range(8):
        jsum[7 * n : 7 * n + 7, n] = 1.0

    def col(x):
        return np.ascontiguousarray(np.asarray(x, dtype=f32).reshape(-1, 1))

    b0bg = np.concatenate([col(inputs["b0"]), col(inputs["bg"])], axis=0)
    bs1_rep = np.concatenate([col(inputs["bs1"])] * 2, axis=0)
    bs2_rep = np.concatenate([col(inputs["bs2"])] * 2, axis=0)

    state0 = a(inputs["state0"]); state1 = a(inputs["state1"])
    state2 = a(inputs["state2"])

    shared = {
        "wg_bf": a(Wg, bf),
        "ws1_blk": a(ws1_blk, bf),
        "ws2_blk": a(ws2_blk, bf),
        "w0": W0,
        "wqk_dup": a(wqk_dup, bf),
        "wc1a": a(Wc1[0:128, :], bf),
        "wvc_dup": a(wvc_dup, bf),
        "wc2": a(Wc2, bf),
        "wc3": a(Wc3, bf),
        "sel": a(sel, bf),
        "scoresel": a(scoresel, bf),
        "ones8": a(ones8, bf),
        "ones18": ones18,
        "jsum": a(jsum, bf),
        "b0bg": b0bg,
        "bs1_rep": bs1_rep,
        "bs2_rep": bs2_rep,
        "bc1": col(inputs["bc1"]),
        "bc2": col(inputs["bc2"]),
        "bc3": col(inputs["bc3"]),
    }
    in_maps = []
    for i in range(N_CORES):
        m = dict(shared)
        sl = slice(i * RPC, (i + 1) * RPC)
        m["state0"] = state0[sl]
        m["state1"] = state1[sl]
        m["state2"] = state2[sl]
        in_maps.append(m)
    return in_maps


_NC_CACHE = {}


def get_nc():
    if "nc" not in _NC_CACHE:
        nc = build()
        nc.finalize()
        _NC_CACHE["nc"] = nc
    return _NC_CACHE["nc"]


def kernel(**inputs):
    nc = get_nc()
    in_maps = prepare_in_maps(inputs)
    trace = bool(int(os.environ.get("K_TRACE", "0")))
    try:
        res = run_bass_kernel_spmd(
            nc, in_maps, core_ids=list(range(N_CORES)), trace=trace
        )
    except ModuleNotFoundError:
        res = run_bass_kernel_spmd(nc, in_maps, core_ids=list(range(N_CORES)))
    if res.exec_time_ns is not None:
        print(f"HW exec time: {res.exec_time_ns} ns")
    parts = [np.asarray(res.results[i]["out"], dtype=np.float32).T for i in range(N_CORES)]
    return np.ascontiguousarray(np.concatenate(parts, axis=0))


# revision 31
# speedup vs baseline: 1.2591x; 1.0258x over previous
"""Trainium2 Bass kernel for nn_ActorNetwork (gnn_message_passing).

Pure data-parallel across 8 NeuronCores: each core processes 8192 of the
65536 batch rows; small weights are replicated.

v2 layout: feature-major throughout, with the attention computed in a fused
(neighbor-pair, macro-column) layout that never transposes i2 back to
batch-major:
  - i1/i2 as 4 wide matmuls each (neighbor-pairs packed on partitions,
    (o,b) = 512 macro columns streamed).
  - score_n[b] = sum_d i2 * q2 via one elementwise multiply (q2 produced
    partition-duplicated for free by a duplicated-column Wq*Wk^T stationary)
    and per-pair ones-matmul partition reductions, accumulated in PSUM on
    top of the PE-transposed -1e30 mask rows -> masked scores directly.
  - softmax over an [8, 512] tile (exp on Act, denominator via ones-matmul,
    reciprocal-dup via 1x8 matmul, one multiply -> alpha).
  - alpha broadcast back to the (r,d) partition layout via 4 selector
    matmuls; weighted i2 (cmul) feeds h1 directly through a row-duplicated
    Wvc stationary -- the attention output is never materialized.
Elementwise/copy work is spread across DVE / Act / Pool; all wide moving
operands are bf16 (DVE 2x mode, PE 1 cyc/col).
"""

import os

import numpy as np
import ml_dtypes

import concourse.bass as bass
import concourse.tile as tile
from concourse import bacc
from concourse import mybir
from concourse.bass_utils import run_bass_kernel_spmd
from concourse.masks import make_identity

F32 = mybir.dt.float32
F32R = mybir.dt.float32r
BF16 = mybir.dt.bfloat16

N_CORES = 8
B_FULL = 65536
RPC = B_FULL // N_CORES        # rows per core = 8192
MACRO = 512                    # batch rows per macro tile
P = 128

Relu = mybir.ActivationFunctionType.Relu
Tanh = mybir.ActivationFunctionType.Tanh
Exp = mybir.ActivationFunctionType.Exp
Alu = mybir.AluOpType
AX = mybir.AxisListType


def build(rpc=RPC, macro=MACRO):
    nm = rpc // macro          # macro tiles per core
    nb = macro // P            # 128-row blocks per macro tile

    nc = bacc.Bacc()

    s0 = nc.declare_dram_parameter("state0", [rpc, 6], F32R, isOutput=False)
    s1 = nc.declare_dram_parameter("state1", [rpc, 1024], F32, isOutput=False)
    s2 = nc.declare_dram_parameter("state2", [rpc, 8, 7], F32, isOutput=False)
    wg_d = nc.declare_dram_parameter("wg_bf", [1024, 64], BF16, isOutput=False)
    ws1_d = nc.declare_dram_parameter("ws1_blk", [56, 4, 128], BF16, isOutput=False)
    ws2_d = nc.declare_dram_parameter("ws2_blk", [128, 128], BF16, isOutput=False)
    w0_d = nc.declare_dram_parameter("w0", [6, 64], F32R, isOutput=False)
    wqk_d = nc.declare_dram_parameter("wqk_dup", [64, 128], BF16, isOutput=False)
    wc1a_d = nc.declare_dram_parameter("wc1a", [128, 128], BF16, isOutput=False)
    wvc_d = nc.declare_dram_parameter("wvc_dup", [128, 128], BF16, isOutput=False)
    wc2_d = nc.declare_dram_parameter("wc2", [128, 128], BF16, isOutput=False)
    wc3_d = nc.declare_dram_parameter("wc3", [128, 2], BF16, isOutput=False)
    sel_d = nc.declare_dram_parameter("sel", [8, 4, 128], BF16, isOutput=False)
    ones2_d = nc.declare_dram_parameter("scoresel", [128, 4, 8], BF16, isOutput=False)
    ones8_d = nc.declare_dram_parameter("ones8", [8, 1], BF16, isOutput=False)
    ones18_d = nc.declare_dram_parameter("ones18", [1, 8], F32R, isOutput=False)
    jsum_d = nc.declare_dram_parameter("jsum", [56, 8], BF16, isOutput=False)
    b0bg_d = nc.declare_dram_parameter("b0bg", [128, 1], F32, isOutput=False)
    bs1_d = nc.declare_dram_parameter("bs1_rep", [128, 1], F32, isOutput=False)
    bs2_d = nc.declare_dram_parameter("bs2_rep", [128, 1], F32, isOutput=False)
    bc1_d = nc.declare_dram_parameter("bc1", [128, 1], F32, isOutput=False)
    bc2_d = nc.declare_dram_parameter("bc2", [128, 1], F32, isOutput=False)
    bc3_d = nc.declare_dram_parameter("bc3", [2, 1], F32, isOutput=False)
    out_d = nc.declare_dram_parameter("out", [2, rpc], F32, isOutput=True)

    with tile.TileContext(nc) as tc:
        consts = tc.alloc_tile_pool(name="consts", bufs=1)
        s1stage_p = tc.alloc_tile_pool(name="s1stage", bufs=2)
        s1T_p = tc.alloc_tile_pool(name="s1T", bufs=2)
        s2_p = tc.alloc_tile_pool(name="s2", bufs=2)
        work_p = tc.alloc_tile_pool(name="work", bufs=3)
        blk_p = tc.alloc_tile_pool(name="blk", bufs=3)
        sm_p = tc.alloc_tile_pool(name="sm", bufs=3)
        psS = tc.alloc_tile_pool(name="psS", bufs=2, space="PSUM")
        psA = tc.alloc_tile_pool(name="psA", bufs=2, space="PSUM")
        psB = tc.alloc_tile_pool(name="psB", bufs=1, space="PSUM")
        psB2 = tc.alloc_tile_pool(name="psB2", bufs=1, space="PSUM")
        psC = tc.alloc_tile_pool(name="psC", bufs=2, space="PSUM")

        # ---- constants / weights to SBUF ----
        wg_sb = consts.tile([P, 8, 64], BF16)
        nc.sync.dma_start(wg_sb, wg_d.rearrange("(c p) m -> p c m", p=P))
        ws1_sb = consts.tile([56, 4, P], BF16)
        nc.sync.dma_start(ws1_sb, ws1_d[:, :, :])
        ws2_sb = consts.tile([P, P], BF16)
        nc.sync.dma_start(ws2_sb, ws2_d[:, :])
        w0_sb = consts.tile([6, 64], F32R)
        nc.sync.dma_start(w0_sb, w0_d[:, :])
        wqk_sb = consts.tile([64, 128], BF16)
        nc.sync.dma_start(wqk_sb, wqk_d[:, :])
        wc1a_sb = consts.tile([P, 128], BF16)
        nc.sync.dma_start(wc1a_sb, wc1a_d[:, :])
        wvc_sb = consts.tile([P, 128], BF16)
        nc.sync.dma_start(wvc_sb, wvc_d[:, :])
        wc2_sb = consts.tile([P, 128], BF16)
        nc.sync.dma_start(wc2_sb, wc2_d[:, :])
        wc3_sb = consts.tile([P, 2], BF16)
        nc.sync.dma_start(wc3_sb, wc3_d[:, :])
        sel_sb = consts.tile([8, 4, P], BF16)
        nc.sync.dma_start(sel_sb, sel_d[:, :, :])
        ones2_sb = consts.tile([P, 4, 8], BF16)
        nc.sync.dma_start(ones2_sb, ones2_d[:, :, :])
        ones8_sb = consts.tile([8, 1], BF16)
        nc.sync.dma_start(ones8_sb, ones8_d[:, :])
        ones18_sb = consts.tile([1, 8], F32R)
        nc.sync.dma_start(ones18_sb, ones18_d[:, :])
        jsum_sb = consts.tile([56, 8], BF16)
        nc.sync.dma_start(jsum_sb, jsum_d[:, :])

        b0bg_sb = consts.tile([P, 1], F32)
        nc.sync.dma_start(b0bg_sb, b0bg_d[:, :])
        bs1_sb = consts.tile([P, 1], F32)
        nc.sync.dma_start(bs1_sb, bs1_d[:, :])
        bs2_sb = consts.tile([P, 1], F32)
        nc.sync.dma_start(bs2_sb, bs2_d[:, :])
        bc1_sb = consts.tile([P, 1], F32)
        nc.sync.dma_start(bc1_sb, bc1_d[:, :])
        bc2_sb = consts.tile([P, 1], F32)
        nc.sync.dma_start(bc2_sb, bc2_d[:, :])
        bc3_sb = consts.tile([2, 1], F32)
        nc.sync.dma_start(bc3_sb, bc3_d[:, :])

        # state0^T loaded once for the whole core (tiny, strided DMA)
        s0T_sb = consts.tile([6, rpc], F32R)
        with nc.allow_non_contiguous_dma(reason="tiny state0 transpose load"):
            nc.sync.dma_start(s0T_sb, s0.rearrange("b f -> f b"))

        ident_f = consts.tile([P, P], F32)
        make_identity(nc, ident_f)
        ident_b = consts.tile([P, P], BF16)
        nc.vector.tensor_copy(ident_b, ident_f)

        out_sb = consts.tile([2, rpc], F32)

        def emit_F1(ms):
            """Loads + state2 pack/transpose + mask rows."""
            row0 = ms * macro
            s1_stage = s1stage_p.tile([P, nb, 1024], BF16, tag="s1stage")
            nc.gpsimd.dma_start(
                s1_stage,
                s1[row0 : row0 + macro, :].rearrange("(o p) f -> p o f", p=P),
            )
            s2_t = s2_p.tile([P, nb, 8, 7], F32, tag="s2")
            nc.sync.dma_start(
                s2_t,
                s2[row0 : row0 + macro, :, :].rearrange("(o p) n j -> p o n j", p=P),
            )
            s2c = sm_p.tile([P, nb, 56], BF16, tag="s2c")
            nc.gpsimd.tensor_copy(s2c, s2_t.rearrange("p o n j -> p o (n j)"))
            s2T_ps = psS.tile([P, nb, P], BF16, tag="psS")
            for o in range(nb):
                nc.tensor.transpose(s2T_ps[0:56, o, :], s2c[:, o, :], ident_b)
            s2T_sb = sm_p.tile([56, nb, P], BF16, tag="s2T")
            nc.vector.tensor_copy(s2T_sb, s2T_ps[0:56])

            nmsum_ps = psC.tile([8, macro], F32, tag="psC")
            nc.tensor.matmul(nmsum_ps, jsum_sb, s2T_sb, start=True, stop=True)
            nm_sb = sm_p.tile([8, macro], BF16, tag="nm")
            nc.vector.tensor_scalar(
                nm_sb, nmsum_ps, 0.0, -1e30, Alu.is_equal, Alu.mult
            )
            return dict(row0=row0, s1_stage=s1_stage, s2T_sb=s2T_sb, nm_sb=nm_sb)

        def emit_F2(st):
            """state1 transposes + env/own + q2."""
            row0 = st["row0"]; s1_stage = st["s1_stage"]
            s1T = s1T_p.tile([P, 8, nb, P], BF16, tag="s1T")
            for o in range(nb):
                t_ps = psS.tile([P, 8, P], BF16, tag="psS")
                for c in range(8):
                    nc.tensor.transpose(
                        t_ps[:, c, :],
                        s1_stage[:, o, c * P : (c + 1) * P],
                        ident_b,
                    )
                dst = s1T[:, :, o, :]
                if o == 1:
                    nc.scalar.copy(dst, t_ps)
                else:
                    nc.vector.tensor_copy(dst, t_ps)

            eo_ps = psB.tile([P, macro], F32, tag="psB")
            nc.tensor.matmul(
                eo_ps[0:64, :], w0_sb, s0T_sb[:, row0 : row0 + macro],
                start=True, stop=True,
            )
            for c in range(8):
                nc.tensor.matmul(
                    eo_ps[64:128, :], wg_sb[:, c], s1T[:, c],
                    start=(c == 0), stop=(c == 7), tile_position=(0, 64),
                )
            concatA = work_p.tile([P, macro], BF16, tag="concatA")
            nc.scalar.activation(concatA, eo_ps, Relu, bias=b0bg_sb)

            q2_ps = psB.tile([P, macro], F32, tag="psB")
            nc.tensor.matmul(q2_ps, wqk_sb, concatA[0:64, :], start=True, stop=True)
            q2_sb = work_p.tile([P, macro], BF16, tag="q2")
            nc.scalar.copy(q2_sb, q2_ps)
            st["concatA"] = concatA
            st["q2_sb"] = q2_sb

        def emit_F3(st):
            """i1, i2, qk."""
            s2T_sb = st["s2T_sb"]; q2_sb = st["q2_sb"]
            i1_sb = blk_p.tile([P, 4, macro], BF16, tag="i1")
            for m in range(4):
                i1_ps = psA.tile([P, macro], F32, tag="psA")
                nc.tensor.matmul(i1_ps, ws1_sb[:, m], s2T_sb, start=True, stop=True)
                dst = i1_sb[:, m, :]
                if m in (0, 3):
                    nc.vector.tensor_scalar(dst, i1_ps, bs1_sb, 0.0, Alu.add, Alu.max)
                else:
                    nc.scalar.activation(dst, i1_ps, Relu, bias=bs1_sb)

            i2_sb = blk_p.tile([P, 4, macro], BF16, tag="i2")
            for m in range(4):
                i2_ps = psA.tile([P, macro], F32, tag="psA")
                nc.tensor.matmul(i2_ps, ws2_sb, i1_sb[:, m, :], start=True, stop=True)
                dst = i2_sb[:, m, :]
                if m in (1, 2):
                    nc.vector.tensor_scalar(dst, i2_ps, bs2_sb, 0.0, Alu.add, Alu.max)
                else:
                    nc.scalar.activation(dst, i2_ps, Relu, bias=bs2_sb)

            qk_sb = blk_p.tile([P, 4, macro], BF16, tag="qk")
            nc.vector.tensor_tensor(
                qk_sb[:, 0:2, :], i2_sb[:, 0:2, :],
                q2_sb[:, None, :].to_broadcast((P, 2, macro)),
                Alu.mult,
            )
            nc.gpsimd.tensor_tensor(
                qk_sb[:, 2:4, :], i2_sb[:, 2:4, :],
                q2_sb[:, None, :].to_broadcast((P, 2, macro)),
                Alu.mult,
            )
            st["i2_sb"] = i2_sb
            st["qk_sb"] = qk_sb

        def emit_B1a(st):
            """Scores -> exp."""
            qk_sb = st["qk_sb"]; nm_sb = st["nm_sb"]
            sc_ps = psC.tile([8, macro], F32, tag="psC")
            for m in range(4):
                nc.tensor.matmul(
                    sc_ps, ones2_sb[:, m, :], qk_sb[:, m, :],
                    start=(m == 0), stop=(m == 3),
                )
            nc.vector.tensor_tensor(sc_ps, sc_ps, nm_sb, Alu.add)
            p8_sb = sm_p.tile([8, macro], BF16, tag="p8")
            nc.scalar.activation(p8_sb, sc_ps, Exp, scale=0.125)
            st["p8_sb"] = p8_sb

        def emit_B1b(st):
            """Softmax denominator -> alpha."""
            p8_sb = st["p8_sb"]
            den_ps = psC.tile([1, macro], F32, tag="psC")
            nc.tensor.matmul(den_ps, ones8_sb, p8_sb, start=True, stop=True)
            rs_sb = sm_p.tile([1, macro], F32R, tag="rs")
            with nc.allow_low_precision(reason="f32r reciprocal, 19-bit ok"):
                nc.vector.reciprocal(rs_sb, den_ps)
            dup_ps = psC.tile([8, macro], F32, tag="psC")
            nc.tensor.matmul(dup_ps, ones18_sb, rs_sb, start=True, stop=True)
            alpha_sb = sm_p.tile([8, macro], BF16, tag="alpha")
            nc.vector.tensor_tensor(alpha_sb, p8_sb, dup_ps, Alu.mult)
            st["alpha_sb"] = alpha_sb

        def emit_B2(st):
            """Weighted i2 + head + tanh."""
            row0 = st["row0"]; concatA = st["concatA"]
            i2_sb = st["i2_sb"]; alpha_sb = st["alpha_sb"]

            cmul_sb = blk_p.tile([P, 4, macro], BF16, tag="cmul")
            abc_sb = sm_p.tile([P, 2, macro], BF16, tag="abc")
            for m in range(4):
                abc_ps = psA.tile([P, macro], F32, tag="psA")
                nc.tensor.matmul(abc_ps, sel_sb[:, m], alpha_sb, start=True, stop=True)
                if m < 2:
                    nc.vector.tensor_tensor(
                        cmul_sb[:, m, :], i2_sb[:, m, :], abc_ps, Alu.mult
                    )
                else:
                    nc.scalar.copy(abc_sb[:, m - 2, :], abc_ps)
            nc.gpsimd.tensor_tensor(
                cmul_sb[:, 2:4, :], i2_sb[:, 2:4, :], abc_sb, Alu.mult
            )

            h1_ps = psB2.tile([P, macro], F32, tag="psB2")
            nc.tensor.matmul(h1_ps, wc1a_sb, concatA, start=True, stop=False)
            for m in range(4):
                nc.tensor.matmul(
                    h1_ps, wvc_sb, cmul_sb[:, m, :],
                    start=False, stop=(m == 3),
                )
            h1_sb = work_p.tile([P, macro], BF16, tag="h1")
            nc.scalar.activation(h1_sb, h1_ps, Relu, bias=bc1_sb)

            h2_ps = psB2.tile([P, macro], F32, tag="psB2")
            nc.tensor.matmul(h2_ps, wc2_sb, h1_sb, start=True, stop=True)
            h2_sb = work_p.tile([P, macro], BF16, tag="h2")
            nc.vector.tensor_scalar(h2_sb, h2_ps, bc2_sb, 0.0, Alu.add, Alu.max)

            o_ps = psB2.tile([2, macro], F32, tag="psB2")
            nc.tensor.matmul(o_ps, wc3_sb, h2_sb, start=True, stop=True)
            nc.scalar.activation(
                out_sb[:, row0 : row0 + macro], o_ps, Tanh, bias=bc3_sb
            )

        # software pipeline, fine-grained: each engine's in-order stream
        # alternates between macro m's latency-critical back half and macro
        # m+1's bulk front half, so dependency stalls are filled.
        prev = None
        for ms in range(nm):
            cur = emit_F1(ms)
            if prev is not None:
                emit_B1a(prev)
            emit_F2(cur)
            if prev is not None:
                emit_B1b(prev)
            emit_F3(cur)
            if prev is not None:
                emit_B2(prev)
            prev = cur
        emit_B1a(prev)
        emit_B1b(prev)
        emit_B2(prev)

        nc.sync.dma_start(out_d[:, :], out_sb)

        for _pool in (psC, psB2, psB, psA, psS, sm_p, blk_p, work_p, s2_p, s1T_p,
                      s1stage_p, consts):
            _pool.release()

    return nc


def prepare_in_maps(inputs):
    bf = ml_dtypes.bfloat16
    f32 = np.float32

    def a(x, dt=f32):
        return np.ascontiguousarray(np.asarray(x), dtype=dt)

    W0 = a(inputs["W0"]); Wg = a(inputs["Wg"])
    Ws1 = a(inputs["Ws1"]); Ws2 = a(inputs["Ws2"])
    Wq = a(inputs["Wq"]); Wk = a(inputs["Wk"]); Wv = a(inputs["Wv"])
    Wc1 = a(inputs["Wc1"]); Wc2 = a(inputs["Wc2"]); Wc3 = a(inputs["Wc3"])

    wqk = Wq @ Wk.T                                   # [64, 64]
    wqk_dup = np.concatenate([wqk, wqk], axis=1)      # [64, 128]
    wvc = Wv @ Wc1[128:192, :]                        # [64, 128]
    wvc_dup = np.concatenate([wvc, wvc], axis=0)      # [128, 128]

    # ws1_blk[7n+j, m, 64r+d] = Ws1[j, d] if n == 2m+r
    ws1_blk = np.zeros((56, 4, 128), dtype=f32)
    sel = np.zeros((8, 4, 128), dtype=f32)
    for n in range(8):
        m, r = n // 2, n % 2
        ws1_blk[7 * n : 7 * n + 7, m, 64 * r : 64 * r + 64] = Ws1
        sel[n, m, 64 * r : 64 * r + 64] = 1.0
    ws2_blk = np.zeros((128, 128), dtype=f32)
    ws2_blk[0:64, 0:64] = Ws2
    ws2_blk[64:128, 64:128] = Ws2

    # scoresel[64r+d, m, n] = 1 iff n == 2m+r (score partition reduce)
    scoresel = np.zeros((128, 4, 8), dtype=f32)
    for n in range(8):
        m, r = n // 2, n % 2
        scoresel[64 * r : 64 * r + 64, m, n] = 1.0
    ones8 = np.ones((8, 1), dtype=f32)
    ones18 = np.ones((1, 8), dtype=f32)
    # jsum[7n+j, n'] = 1 iff n == n'  (per-neighbor feature sums for the mask)
    jsum = np.zeros((56, 8), dtype=f32)
    for n in range(8):
        jsum[7 * n : 7 * n + 7, n] = 1.0

    def col(x):
        return np.ascontiguousarray(np.asarray(x, dtype=f32).reshape(-1, 1))

    b0bg = np.concatenate([col(inputs["b0"]), col(inputs["bg"])], axis=0)
    bs1_rep = np.concatenate([col(inputs["bs1"])] * 2, axis=0)
    bs2_rep = np.concatenate([col(inputs["bs2"])] * 2, axis=0)

    state0 = a(inputs["state0"]); state1 = a(inputs["state1"])
    state2 = a(inputs["state2"])

    shared = {
        "wg_bf": a(Wg, bf),
        "ws1_blk": a(ws1_blk, bf),
        "ws2_blk": a(ws2_blk, bf),
        "w0": W0,
        "wqk_dup": a(wqk_dup, bf),
        "wc1a": a(Wc1[0:128, :], bf),
        "wvc_dup": a(wvc_dup, bf),
        "wc2": a(Wc2, bf),
        "wc3": a(Wc3, bf),
        "sel": a(sel, bf),
        "scoresel": a(scoresel, bf),
        "ones8": a(ones8, bf),
        "ones18": ones18,
        "jsum": a(jsum, bf),
        "b0bg": b0bg,
        "bs1_rep": bs1_rep,
        "bs2_rep": bs2_rep,
        "bc1": col(inputs["bc1"]),
        "bc2": col(inputs["bc2"]),
        "bc3": col(inputs["bc3"]),
    }
    in_maps = []
    for i in range(N_CORES):
        m = dict(shared)
        sl = slice(i * RPC, (i + 1) * RPC)
        m["state0"] = state0[sl]
        m["state1"] = state1[sl]
        m["state2"] = state2[sl]
        in_maps.append(m)
    return in_maps


_NC_CACHE = {}


def get_nc():
    if "nc" not in _NC_CACHE:
        nc = build()
        nc.finalize()
        _NC_CACHE["nc"] = nc
    return _NC_CACHE["nc"]


def kernel(**inputs):
    nc = get_nc()
    in_maps = prepare_in_maps(inputs)
    trace = bool(int(os.environ.get("K_TRACE", "0")))
    try:
        res = run_bass_kernel_spmd(
            nc, in_maps, core_ids=list(range(N_CORES)), trace=trace
        )
    except ModuleNotFoundError:
        res = run_bass_kernel_spmd(nc, in_maps, core_ids=list(range(N_CORES)))
    if res.exec_time_ns is not None:
        print(f"HW exec time: {res.exec_time_ns} ns")
    parts = [np.asarray(res.results[i]["out"], dtype=np.float32).T for i in range(N_CORES)]
    return np.ascontiguousarray(np.concatenate(parts, axis=0))
